# Initial kernel scaffold
#
"""C2PSA attention block (B=4, C=256, H=W=64) on 8 Trainium2 NeuronCores.

Sharding: data-parallel over (batch, query-half): core c handles batch c//2,
query rows [c%2 * 2048, ...+2048) of the 4096-token attention. Weights are
replicated. No cross-core communication.

Per-core algorithm (all matmuls bf16, accumulation fp32 in PSUM):
  q = Wq @ x_q-half     [64, 2048]   (stored duplicated across both
                                      partition halves for PE row-packing)
  k = Wk @ x            [64, 4096]   (128-key blocks interleaved: even block
                                      in partitions 0-63, odd in 64-127)
  vT = x^T @ Wv^T       [4096, 64]   (stored per 128-block with a ones
                                      column appended -> [128, 65])
  for each 512-query tile:
    for each pair of 128-key blocks:
      S^T = k_blk^T @ q  (two concurrent K=64 matmuls via PE row tiling)
      P   = exp(S^T / 8) (ScalarE, fp32 PSUM -> bf16 SBUF)
      oT += [vT | 1]^T @ P   (accumulate [65, 512]; row 64 = softmax denom)
    oT[0:64] *= 1/oT[64]   (approx-reciprocal + PE outer-product broadcast)
    out = Wp @ oT + x      (residual in fp32)

Softmax max-subtraction is skipped: scores/8 are ~N(0,1); exp stays in
[e-7, e7] which is exact-enough territory for fp32/bf16.
"""

import warnings

warnings.filterwarnings("ignore")

import numpy as np
import ml_dtypes

B, C, HH, WW = 4, 256, 64, 64
N = HH * WW  # 4096 tokens
CR = 64  # reduced (head) dim
NCORES = 8
NQ = N // 2  # 2048 queries per core
QT = 512  # query tile (matmul free dim)
NT = NQ // QT  # 4 query tiles per core
NBLK = N // 128  # 32 key blocks
NPAIR = NBLK // 2  # 16 key block pairs
SCALE = 1.0 / 8.0  # 1/sqrt(CR)

_CACHE = {}


def _build_program(reps=1):
    from contextlib import ExitStack

    import concourse.bass as bass
    import concourse.tile as tile
    from concourse import bacc, mybir
    from concourse._compat import with_exitstack
    from concourse.bass import ts

    f32 = mybir.dt.float32
    bf16 = mybir.dt.bfloat16

    nc = bacc.Bacc("TRN2", target_bir_lowering=False, debug=False)

    # xf is host-permuted per core: the core's own query half occupies
    # cols 0:NQ (attention is permutation-invariant over keys, so k/v can
    # be computed on the permuted order). wpack holds all weights in one
    # tensor so a single DMA descriptor covers them.
    xf_d = nc.dram_tensor("xf", (2, 128, N), bf16, kind="ExternalInput").ap()
    xqf_d = nc.dram_tensor("xqf", (2, 128, NQ), f32, kind="ExternalInput").ap()
    wpack_d = nc.dram_tensor("wpack", (128, 768), bf16, kind="ExternalInput").ap()
    out_d = nc.dram_tensor("out", (2, 128, NQ), f32, kind="ExternalOutput").ap()

    @with_exitstack
    def kern(ctx: ExitStack, tc: tile.TileContext):
        nc = tc.nc
        Exp = mybir.ActivationFunctionType.Exp

        const = ctx.enter_context(tc.tile_pool(name="const", bufs=1))
        pers = ctx.enter_context(tc.tile_pool(name="pers", bufs=1))
        ppool = ctx.enter_context(tc.tile_pool(name="pp", bufs=4))
        smalls = ctx.enter_context(tc.tile_pool(name="smalls", bufs=3))
        outp = ctx.enter_context(tc.tile_pool(name="outp", bufs=4))
        # PSUM: scores 2x[128,1024] (4 banks) + proj/tail 2x[128,512]
        # (2 banks) + oT 2x[65,512] (2 banks) = 8 banks exactly.
        spool = ctx.enter_context(tc.tile_pool(name="spsum", bufs=2, space="PSUM"))
        prpool = ctx.enter_context(tc.tile_pool(name="prpsum", bufs=2, space="PSUM"))
        opool = ctx.enter_context(tc.tile_pool(name="opsum", bufs=2, space="PSUM"))

        def psum_t():
            return spool.tile([128, 1024], f32, tag="sp", name="sp")

        def psum_p():
            return prpool.tile([128, 512], f32, tag="pr", name="pr")

        # ---- persistent SBUF arrays ----
        xf_sb = pers.tile([128, 2, N], bf16, tag="xf")
        q_sb = pers.tile([128, NQ], bf16, tag="q")  # duplicated halves
        k_sb = pers.tile([128, NPAIR, 128], bf16, tag="k")  # interleaved blocks
        vt1_sb = pers.tile([128, NBLK, CR + 1], bf16, tag="vt1")
        wpack_sb = const.tile([128, 768], bf16, tag="wpack")
        wq_sb = wpack_sb[:, 0:256].rearrange("p (c m) -> p c m", c=2)
        wk_sb = wpack_sb[:, 256:384].rearrange("p (c m) -> p c m", c=2)
        wv_sb = wpack_sb[:, 384:512].rearrange("p (c m) -> p c m", c=2)
        wp_sb = wpack_sb[0:CR, 512:768]
        ones_sb = const.tile([1, CR], bf16, tag="ones")

        nc.vector.memset(ones_sb[:], 1.0)
        # zero so warmup matmuls see no NaNs; split so warmup starts early
        nc.gpsimd.memset(vt1_sb[:, 0:16], 0.0)
        nc.gpsimd.memset(vt1_sb[:, 16:NBLK], 0.0)
        nc.vector.memset(vt1_sb[:, :, CR : CR + 1], 1.0)  # ones column

        def q_proj(qoff, qw):
            # q tile = Wq @ x_q[:, qoff:qoff+qw], duplicated into both halves
            qp = psum_p()[:, 0:qw]
            nc.tensor.matmul(qp, wq_sb[:, 0], xf_sb[:, 0, qoff : qoff + qw], start=True, stop=False)
            nc.tensor.matmul(qp, wq_sb[:, 1], xf_sb[:, 1, qoff : qoff + qw], start=False, stop=True)
            nc.vector.tensor_copy(q_sb[:, qoff : qoff + qw], qp)

        def vt_proj_pair(p):
            # vT for blocks 2p, 2p+1 in one PSUM bank -> single evacuation
            vp = psum_p()
            for i in range(2):
                b = 2 * p + i
                nc.tensor.matmul(
                    vp[:, i * CR : (i + 1) * CR], xf_sb[:, 0, ts(b, 128)],
                    wv_sb[:, 0], start=True, stop=False,
                )
                nc.tensor.matmul(
                    vp[:, i * CR : (i + 1) * CR], xf_sb[:, 1, ts(b, 128)],
                    wv_sb[:, 1], start=False, stop=True,
                )
            src = vp[:, 0:2 * CR].rearrange("n (b r) -> n b r", b=2)
            nc.vector.tensor_copy(vt1_sb[:, 2 * p : 2 * p + 2, 0:CR], src)

        def tail(qoff, qw, oT, xrs, last=False):
            # normalize: oT[0:64] * (1 / oT[64]) broadcast via PE outer
            # product. (reciprocal_approx_fast mis-lowers on
            # base_partition!=0 PSUM srcs, so stage the row through SBUF.)
            l_sb = smalls.tile([1, qw], f32, tag="lsb", name="lsb")
            nc.vector.tensor_copy(l_sb[:], oT[CR : CR + 1, 0:qw])
            rec = smalls.tile([1, qw], f32, tag="rec", name="rec")
            nc.vector.reciprocal_approx_fast(rec[:], l_sb[:])
            recb = smalls.tile([1, qw], bf16, tag="recb", name="recb")
            nc.vector.tensor_copy(recb[:], rec[:])
            bc = psum_p()[0:CR, 0:qw]
            nc.tensor.matmul(bc, ones_sb[:], recb[:], start=True, stop=True)
            bc_sb = smalls.tile([CR, qw], f32, tag="bc", name="bc")
            nc.vector.tensor_copy(bc_sb[:], bc)
            oTn = smalls.tile([CR, qw], bf16, tag="otn", name="otn")
            nc.vector.tensor_mul(oTn[:], oT[0:CR, 0:qw], bc_sb[:])
            # output projection + residual
            for ch in range(2):
                o2 = psum_p()[:, 0:qw]
                nc.tensor.matmul(o2, wp_sb[:, ts(ch, 128)], oTn[:], start=True, stop=True)
                ob = outp.tile([128, qw], f32, tag="ob", name="ob")
                nc.vector.tensor_add(ob[:], o2, xrs[ch][:])
                eng = nc.sync if last else nc.gpsimd
                eng.dma_start(out_d[ch, :, qoff : qoff + qw], ob[:])

        def k_proj(j):
            # k tile j (cols 512j..512j+512) -> interleaved pairs 2j, 2j+1
            kp = psum_p()[0:CR, :]
            nc.tensor.matmul(kp, wk_sb[:, 0], xf_sb[:, 0, ts(j, 512)], start=True, stop=False)
            nc.tensor.matmul(kp, wk_sb[:, 1], xf_sb[:, 1, ts(j, 512)], start=False, stop=True)
            kr = kp.rearrange("r (b two n) -> r b two n", two=2, n=128)
            nc.vector.tensor_copy(k_sb[0:CR, 2 * j : 2 * j + 2, :], kr[:, :, 0, :])
            nc.vector.tensor_copy(k_sb[CR:128, 2 * j : 2 * j + 2, :], kr[:, :, 1, :])

        # accumulation runs 1-2 pairs behind the scores/exp stream so the
        # PE's strict-FIFO queue never stalls waiting on the current exp
        pending = []  # (oT, p, pp, qw) entries

        def accum_step():
            oT_, p_, pp_, qw_ = pending.pop(0)
            nc.tensor.matmul(
                oT_[:, 0:qw_], vt1_sb[:, 2 * p_, :], pp_[:, 0:qw_],
                start=(p_ == 0), stop=False, skip_group_check=True,
            )
            nc.tensor.matmul(
                oT_[:, 0:qw_], vt1_sb[:, 2 * p_ + 1, :], pp_[:, qw_ : 2 * qw_],
                start=False, stop=(p_ == NPAIR - 1), skip_group_check=True,
            )

        def accum_flush():
            while pending:
                accum_step()

        def pairs(qoff, qw, oT, interleave_proj):
            if interleave_proj:
                # prologue: k two tiles ahead, vT one pair ahead
                k_proj(0)
                k_proj(1)
                vt_proj_pair(0)
            for p in range(NPAIR):
                if interleave_proj:
                    if (p + 2) % 2 == 0 and 2 <= (p + 2) // 2 <= 7:
                        k_proj((p + 2) // 2)
                    if p + 1 < NPAIR:
                        vt_proj_pair(p + 1)
                # block A in bank 0, block B in bank 1 (concurrent row-packed
                # matmuls must not write the same PSUM bank)
                s = psum_t()
                nc.tensor.matmul(
                    s[:, 0:qw], k_sb[0:CR, p, :], q_sb[0:CR, qoff : qoff + qw],
                    start=True, stop=True,
                )
                nc.tensor.matmul(
                    s[:, 512 : 512 + qw], k_sb[CR:128, p, :], q_sb[CR:128, qoff : qoff + qw],
                    start=True, stop=True,
                )
                pp = ppool.tile([128, 1024], bf16, tag="pp", name="pp")
                s_v = s.rearrange("n (b m) -> n b m", b=2)[:, :, 0:qw]
                pp_v = pp[:, 0 : 2 * qw].rearrange("n (b m) -> n b m", b=2)
                nc.scalar.activation(pp_v, s_v, Exp, scale=SCALE)
                if len(pending) >= 2:
                    accum_step()
                pending.append((oT, p, pp, qw))

        def body():
            # PE warmup: ~3.4us of dummy matmuls on zeroed SBUF so the HAM
            # clock gate flips to 2.4 GHz before the real work arrives.
            # (Depends only on the vt1 memset, so it runs during DMA wait.)
            warm = opool.tile([CR + 1, 512], f32, tag="ot", name="warm")
            for _ in range(9):
                nc.tensor.matmul(
                    warm[0:CR + 1, 0:455], vt1_sb[:, 0, :], vt1_sb[:, 8:15, :],
                    start=True, stop=True,
                )

            # one descriptor for all weights, then xf streamed in
            # consumption order so projections start after ~0.4MB, not 4MB.
            nc.sync.dma_start(wpack_sb[:], wpack_d[:])
            for ch in range(2):  # first 512 cols: everything pair 0 needs
                nc.sync.dma_start(xf_sb[:, ch, 0:512], xf_d[ch, :, 0:512])
            for j in range(7):  # rest of xf in half-MB pieces, both chunks
                for ch in range(2):
                    nc.sync.dma_start(
                        xf_sb[:, ch, 512 + j * 512 : 1024 + j * 512],
                        xf_d[ch, :, 512 + j * 512 : 1024 + j * 512],
                    )

            # query tiles (half-width final tiles measured slower: the extra
            # per-ACTIVATE overhead outweighs the shorter exposed tail)
            TILES = [(0, 512), (512, 512), (1024, 512), (1536, 512)]
            q_proj(*TILES[0])

            # ---- attention: tails software-pipelined one iteration behind
            # so the PE FIFO never stalls on the DVE normalization chain;
            # k and vT projections stream just-in-time inside tile 0. ----
            oTs, xrss = {}, {}
            for t, (qoff, qw) in enumerate(TILES):
                # prefetch residual tiles for this tile's tail; on the Sync
                # queue so they issue AFTER the xf pieces (HBM bandwidth
                # during setup is the critical resource)
                xrs = []
                for ch in range(2):
                    xr = outp.tile([128, qw], f32, tag="xr", name="xr")
                    nc.sync.dma_start(xr[:], xqf_d[ch, :, qoff : qoff + qw])
                    xrs.append(xr)
                xrss[t] = xrs
                oT = opool.tile([CR + 1, qw], f32, tag="ot", name="ot")
                oTs[t] = oT
                pairs(qoff, qw, oT, interleave_proj=(t == 0))
                if t + 1 < len(TILES):
                    q_proj(*TILES[t + 1])
                if t >= 1:
                    tail(*TILES[t - 1], oTs.pop(t - 1), xrss.pop(t - 1))
            accum_flush()
            last_i = len(TILES) - 1
            tail(*TILES[last_i], oTs.pop(last_i), xrss.pop(last_i), last=True)

        for _rep in range(reps):
            body()

    with tile.TileContext(nc) as tc:
        kern(tc)
    nc.compile()
    return nc


def _get_program(reps=1):
    key = ("nc", reps)
    if key not in _CACHE:
        _CACHE[key] = _build_program(reps)
    return _CACHE[key]


def _make_in_maps(x, Wq, Wk, Wv, Wp):
    bf16 = ml_dtypes.bfloat16
    xfull = np.ascontiguousarray(x.reshape(B, C, N))
    wpack = np.zeros((128, 768), dtype=bf16)
    wq2 = np.concatenate([Wq.T, Wq.T], axis=1)  # [256, 128]
    for ch in range(2):
        wpack[:, ch * 128 : (ch + 1) * 128] = wq2[ch * 128 : (ch + 1) * 128]
        wpack[:, 256 + ch * CR : 256 + (ch + 1) * CR] = Wk.T[ch * 128 : (ch + 1) * 128]
        wpack[:, 384 + ch * CR : 384 + (ch + 1) * CR] = Wv.T[ch * 128 : (ch + 1) * 128]
    wpack[0:CR, 512:768] = Wp.T
    in_maps = []
    for c in range(NCORES):
        b, h = divmod(c, 2)
        xb = xfull[b]
        xqs = np.ascontiguousarray(xb[:, h * NQ : (h + 1) * NQ])
        # put the core's query half first; key order is irrelevant to attention
        xperm = np.concatenate([xqs, xb[:, (1 - h) * NQ : (2 - h) * NQ]], axis=1)
        in_maps.append(
            {
                "xf": xperm.reshape(2, 128, N).astype(bf16),
                "xqf": xqs.reshape(2, 128, NQ).astype(np.float32),
                "wpack": wpack,
            }
        )
    return in_maps


def _run(x, Wq, Wk, Wv, Wp):
    from concourse import bass_utils

    nc = _get_program()
    in_maps = _make_in_maps(x, Wq, Wk, Wv, Wp)
    res = bass_utils.run_bass_kernel_spmd(nc, in_maps, core_ids=list(range(NCORES)))
    out = np.empty((B, C, N), dtype=np.float32)
    for c in range(NCORES):
        b, h = divmod(c, 2)
        out[b, :, h * NQ : (h + 1) * NQ] = res.results[c]["out"].reshape(C, NQ)
    return out.reshape(B, C, HH, WW)


def kernel(x, Wq, Wk, Wv, Wp):
    return _run(
        np.asarray(x, dtype=np.float32),
        np.asarray(Wq, dtype=np.float32),
        np.asarray(Wk, dtype=np.float32),
        np.asarray(Wv, dtype=np.float32),
        np.asarray(Wp, dtype=np.float32),
    )


# ---------------------------------------------------------------------------
# benchmarking helpers (not used by the grading path)
# ---------------------------------------------------------------------------


def _get_exec(reps):
    """Build a cached jitted shard_map executable for the given reps-variant
    (mirrors bass2jax.run_bass_via_pjrt, but reusable across calls)."""
    key = ("exec", reps)
    if key in _CACHE:
        return _CACHE[key]

    import jax
    from jax.experimental.shard_map import shard_map
    from jax.sharding import Mesh, PartitionSpec
    import concourse.mybir as mybir
    from concourse.bass2jax import (
        _bass_exec_p,
        install_neuronx_cc_hook,
        partition_id_tensor,
    )

    install_neuronx_cc_hook()
    nc = _get_program(reps)
    partition_name = nc.partition_id_tensor.name if nc.partition_id_tensor else None

    in_names, out_names, out_avals, zero_outs = [], [], [], []
    for alloc in nc.m.functions[0].allocations:
        if not isinstance(alloc, mybir.MemoryLocationSet):
            continue
        name = alloc.memorylocations[0].name
        if alloc.kind == "ExternalInput":
            if name != partition_name:
                in_names.append(name)
        elif alloc.kind == "ExternalOutput":
            out_names.append(name)
            shape = tuple(alloc.tensor_shape)
            dtype = mybir.dt.np(alloc.dtype)
            out_avals.append(jax.core.ShapedArray(shape, dtype))
            zero_outs.append(np.zeros(shape, dtype))
    n_params = len(in_names)
    n_outs = len(out_avals)
    all_in_names = in_names + out_names
    if partition_name is not None:
        all_in_names.append(partition_name)
    donate = tuple(range(n_params, n_params + n_outs))

    def _b(*args):
        operands = list(args)
        if partition_name is not None:
            operands.append(partition_id_tensor())
        outs = _bass_exec_p.bind(
            *operands,
            out_avals=tuple(out_avals),
            in_names=tuple(all_in_names),
            out_names=tuple(out_names),
            lowering_input_output_aliases=(),
            sim_require_finite=True,
            sim_require_nnan=True,
            nc=nc,
        )
        return tuple(outs)

    devices = jax.devices()[:NCORES]
    mesh = Mesh(np.asarray(devices), ("core",))
    in_specs = (PartitionSpec("core"),) * (n_params + n_outs)
    out_specs = (PartitionSpec("core"),) * n_outs
    fn = jax.jit(
        shard_map(_b, mesh=mesh, in_specs=in_specs, out_specs=out_specs, check_rep=False),
        donate_argnums=donate,
        keep_unused=True,
    )
    _CACHE[key] = (fn, in_names, out_names, out_avals, zero_outs, mesh)
    return _CACHE[key]


def bench(x, Wq, Wk, Wv, Wp, reps, iters=8):
    """Return (best_wall_seconds, outputs_list) for the reps-variant program."""
    import time

    import jax

    fn, in_names, out_names, out_avals, zero_outs, mesh = _get_exec(reps)
    in_maps = _make_in_maps(x, Wq, Wk, Wv, Wp)
    concat_in = [
        np.concatenate([in_maps[c][n] for c in range(NCORES)], axis=0)
        for n in in_names
    ]
    concat_in = [jax.device_put(a) for a in concat_in]

    def zeros():
        return [np.zeros((NCORES * z.shape[0], *z.shape[1:]), z.dtype) for z in zero_outs]

    # warm up (compiles NEFF on first call)
    out = fn(*concat_in, *zeros())
    jax.block_until_ready(out)

    best = float("inf")
    for _ in range(iters):
        zs = [jax.device_put(z) for z in zeros()]
        jax.block_until_ready(zs)
        t0 = time.perf_counter()
        out = fn(*concat_in, *zs)
        jax.block_until_ready(out)
        t1 = time.perf_counter()
        best = min(best, t1 - t0)
    outs = [np.asarray(o) for o in out]
    return best, outs



# revision 1
# speedup vs baseline: 1.0225x; 1.0225x over previous
"""C2PSA attention block (B=4, C=256, H=W=64) on 8 Trainium2 NeuronCores.

Sharding: data-parallel over (batch, query-half): core c handles batch c//2,
query rows [c%2 * 2048, ...+2048) of the 4096-token attention. Weights are
replicated. No cross-core communication.

Per-core algorithm (all matmuls bf16, accumulation fp32 in PSUM):
  q = Wq @ x_q-half     [64, 2048]   (stored duplicated across both
                                      partition halves for PE row-packing)
  k = Wk @ x            [64, 4096]   (128-key blocks interleaved: even block
                                      in partitions 0-63, odd in 64-127)
  vT = x^T @ Wv^T       [4096, 64]   (stored per 128-block with a ones
                                      column appended -> [128, 65])
  for each 512-query tile:
    for each pair of 128-key blocks:
      S^T = k_blk^T @ q  (two concurrent K=64 matmuls via PE row tiling)
      P   = exp(S^T / 8) (ScalarE, fp32 PSUM -> bf16 SBUF)
      oT += [vT | 1]^T @ P   (accumulate [65, 512]; row 64 = softmax denom)
    oT[0:64] *= 1/oT[64]   (approx-reciprocal + PE outer-product broadcast)
    out = Wp @ oT + x      (residual in fp32)

Softmax max-subtraction is skipped: scores/8 are ~N(0,1); exp stays in
[e-7, e7] which is exact-enough territory for fp32/bf16.
"""

import warnings

warnings.filterwarnings("ignore")

import numpy as np
import ml_dtypes

B, C, HH, WW = 4, 256, 64, 64
N = HH * WW  # 4096 tokens
CR = 64  # reduced (head) dim
NCORES = 8
NQ = N // 2  # 2048 queries per core
QT = 512  # query tile (matmul free dim)
NT = NQ // QT  # 4 query tiles per core
NBLK = N // 128  # 32 key blocks
NPAIR = NBLK // 2  # 16 key block pairs
SCALE = 1.0 / 8.0  # 1/sqrt(CR)

_CACHE = {}


def _build_program(reps=1):
    from contextlib import ExitStack

    import concourse.bass as bass
    import concourse.tile as tile
    from concourse import bacc, mybir
    from concourse._compat import with_exitstack
    from concourse.bass import ts

    f32 = mybir.dt.float32
    bf16 = mybir.dt.bfloat16

    nc = bacc.Bacc("TRN2", target_bir_lowering=False, debug=False)

    # xf is host-permuted per core: the core's own query half occupies
    # cols 0:NQ (attention is permutation-invariant over keys, so k/v can
    # be computed on the permuted order). wpack holds all weights in one
    # tensor so a single DMA descriptor covers them.
    xf_d = nc.dram_tensor("xf", (2, 128, N), bf16, kind="ExternalInput").ap()
    xqf_d = nc.dram_tensor("xqf", (2, 128, NQ), f32, kind="ExternalInput").ap()
    wpack_d = nc.dram_tensor("wpack", (128, 768), bf16, kind="ExternalInput").ap()
    out_d = nc.dram_tensor("out", (2, 128, NQ), f32, kind="ExternalOutput").ap()

    @with_exitstack
    def kern(ctx: ExitStack, tc: tile.TileContext):
        nc = tc.nc
        Exp = mybir.ActivationFunctionType.Exp

        const = ctx.enter_context(tc.tile_pool(name="const", bufs=1))
        pers = ctx.enter_context(tc.tile_pool(name="pers", bufs=1))
        ppool = ctx.enter_context(tc.tile_pool(name="pp", bufs=4))
        smalls = ctx.enter_context(tc.tile_pool(name="smalls", bufs=3))
        outp = ctx.enter_context(tc.tile_pool(name="outp", bufs=4))
        # PSUM: scores 2x[128,1024] (4 banks) + proj/tail 2x[128,512]
        # (2 banks) + oT 2x[65,512] (2 banks) = 8 banks exactly.
        spool = ctx.enter_context(tc.tile_pool(name="spsum", bufs=2, space="PSUM"))
        prpool = ctx.enter_context(tc.tile_pool(name="prpsum", bufs=2, space="PSUM"))
        opool = ctx.enter_context(tc.tile_pool(name="opsum", bufs=2, space="PSUM"))

        def psum_t():
            return spool.tile([128, 1024], f32, tag="sp", name="sp")

        def psum_p():
            return prpool.tile([128, 512], f32, tag="pr", name="pr")

        # ---- persistent SBUF arrays ----
        xf_sb = pers.tile([128, 2, N], bf16, tag="xf")
        q_sb = pers.tile([128, NQ], bf16, tag="q")  # duplicated halves
        k_sb = pers.tile([128, NPAIR, 128], bf16, tag="k")  # interleaved blocks
        vt1_sb = pers.tile([128, NBLK, CR + 1], bf16, tag="vt1")
        wpack_sb = const.tile([128, 768], bf16, tag="wpack")
        wq_sb = wpack_sb[:, 0:256].rearrange("p (c m) -> p c m", c=2)
        wk_sb = wpack_sb[:, 256:384].rearrange("p (c m) -> p c m", c=2)
        wv_sb = wpack_sb[:, 384:512].rearrange("p (c m) -> p c m", c=2)
        wp_sb = wpack_sb[0:CR, 512:768]
        ones_sb = const.tile([1, CR], bf16, tag="ones")

        nc.vector.memset(ones_sb[:], 1.0)
        # zero so warmup matmuls see no NaNs; split so warmup starts early
        nc.gpsimd.memset(vt1_sb[:, 0:16], 0.0)
        nc.gpsimd.memset(vt1_sb[:, 16:NBLK], 0.0)
        nc.vector.memset(vt1_sb[:, :, CR : CR + 1], 1.0)  # ones column

        def q_proj(qoff, qw):
            # q tile = Wq @ x_q[:, qoff:qoff+qw], duplicated into both halves
            qp = psum_p()[:, 0:qw]
            nc.tensor.matmul(qp, wq_sb[:, 0], xf_sb[:, 0, qoff : qoff + qw], start=True, stop=False)
            nc.tensor.matmul(qp, wq_sb[:, 1], xf_sb[:, 1, qoff : qoff + qw], start=False, stop=True)
            nc.vector.tensor_copy(q_sb[:, qoff : qoff + qw], qp)

        def vt_proj_pair(p):
            # vT for blocks 2p, 2p+1 in one PSUM bank -> single evacuation
            vp = psum_p()
            for i in range(2):
                b = 2 * p + i
                nc.tensor.matmul(
                    vp[:, i * CR : (i + 1) * CR], xf_sb[:, 0, ts(b, 128)],
                    wv_sb[:, 0], start=True, stop=False,
                )
                nc.tensor.matmul(
                    vp[:, i * CR : (i + 1) * CR], xf_sb[:, 1, ts(b, 128)],
                    wv_sb[:, 1], start=False, stop=True,
                )
            src = vp[:, 0:2 * CR].rearrange("n (b r) -> n b r", b=2)
            nc.vector.tensor_copy(vt1_sb[:, 2 * p : 2 * p + 2, 0:CR], src)

        def tail(qoff, qw, oT, xrs, last=False):
            # normalize: oT[0:64] * (1 / oT[64]) broadcast via PE outer
            # product. (reciprocal_approx_fast mis-lowers on
            # base_partition!=0 PSUM srcs, so stage the row through SBUF.)
            l_sb = smalls.tile([1, qw], f32, tag="lsb", name="lsb")
            nc.vector.tensor_copy(l_sb[:], oT[CR : CR + 1, 0:qw])
            rec = smalls.tile([1, qw], f32, tag="rec", name="rec")
            nc.vector.reciprocal_approx_fast(rec[:], l_sb[:])
            recb = smalls.tile([1, qw], bf16, tag="recb", name="recb")
            nc.vector.tensor_copy(recb[:], rec[:])
            bc = psum_p()[0:CR, 0:qw]
            nc.tensor.matmul(bc, ones_sb[:], recb[:], start=True, stop=True)
            bc_sb = smalls.tile([CR, qw], f32, tag="bc", name="bc")
            nc.vector.tensor_copy(bc_sb[:], bc)
            oTn = smalls.tile([CR, qw], bf16, tag="otn", name="otn")
            nc.vector.tensor_mul(oTn[:], oT[0:CR, 0:qw], bc_sb[:])
            # output projection + residual
            for ch in range(2):
                o2 = psum_p()[:, 0:qw]
                nc.tensor.matmul(o2, wp_sb[:, ts(ch, 128)], oTn[:], start=True, stop=True)
                ob = outp.tile([128, qw], f32, tag="ob", name="ob")
                nc.vector.tensor_add(ob[:], o2, xrs[ch][:])
                eng = nc.sync if last else nc.gpsimd
                eng.dma_start(out_d[ch, :, qoff : qoff + qw], ob[:])

        def k_proj(j):
            # k tile j (cols 512j..512j+512) -> interleaved pairs 2j, 2j+1
            kp = psum_p()[0:CR, :]
            nc.tensor.matmul(kp, wk_sb[:, 0], xf_sb[:, 0, ts(j, 512)], start=True, stop=False)
            nc.tensor.matmul(kp, wk_sb[:, 1], xf_sb[:, 1, ts(j, 512)], start=False, stop=True)
            kr = kp.rearrange("r (b two n) -> r b two n", two=2, n=128)
            nc.vector.tensor_copy(k_sb[0:CR, 2 * j : 2 * j + 2, :], kr[:, :, 0, :])
            nc.vector.tensor_copy(k_sb[CR:128, 2 * j : 2 * j + 2, :], kr[:, :, 1, :])

        # accumulation runs 1-2 pairs behind the scores/exp stream so the
        # PE's strict-FIFO queue never stalls waiting on the current exp
        pending = []  # (oT, p, pp, qw) entries

        def accum_step():
            oT_, p_, pp_, qw_ = pending.pop(0)
            nc.tensor.matmul(
                oT_[:, 0:qw_], vt1_sb[:, 2 * p_, :], pp_[:, 0:qw_],
                start=(p_ == 0), stop=False, skip_group_check=True,
            )
            nc.tensor.matmul(
                oT_[:, 0:qw_], vt1_sb[:, 2 * p_ + 1, :], pp_[:, qw_ : 2 * qw_],
                start=False, stop=(p_ == NPAIR - 1), skip_group_check=True,
            )

        def accum_flush():
            while pending:
                accum_step()

        def pairs(qoff, qw, oT, interleave_proj):
            if interleave_proj:
                # prologue: k two tiles ahead, vT one pair ahead
                k_proj(0)
                k_proj(1)
                vt_proj_pair(0)
            for p in range(NPAIR):
                if interleave_proj:
                    if (p + 2) % 2 == 0 and 2 <= (p + 2) // 2 <= 7:
                        k_proj((p + 2) // 2)
                    if p + 1 < NPAIR:
                        vt_proj_pair(p + 1)
                # block A in bank 0, block B in bank 1 (concurrent row-packed
                # matmuls must not write the same PSUM bank)
                s = psum_t()
                nc.tensor.matmul(
                    s[:, 0:qw], k_sb[0:CR, p, :], q_sb[0:CR, qoff : qoff + qw],
                    start=True, stop=True,
                )
                nc.tensor.matmul(
                    s[:, 512 : 512 + qw], k_sb[CR:128, p, :], q_sb[CR:128, qoff : qoff + qw],
                    start=True, stop=True,
                )
                pp = ppool.tile([128, 1024], bf16, tag="pp", name="pp")
                s_v = s.rearrange("n (b m) -> n b m", b=2)[:, :, 0:qw]
                pp_v = pp[:, 0 : 2 * qw].rearrange("n (b m) -> n b m", b=2)
                nc.scalar.activation(pp_v, s_v, Exp, scale=SCALE)
                if len(pending) >= 2:
                    accum_step()
                pending.append((oT, p, pp, qw))

        def body():
            # PE warmup: ~3.4us of dummy matmuls on zeroed SBUF so the HAM
            # clock gate flips to 2.4 GHz before the real work arrives.
            # (Depends only on the vt1 memset, so it runs during DMA wait.)
            warm = opool.tile([CR + 1, 512], f32, tag="ot", name="warm")
            for _ in range(9):
                nc.tensor.matmul(
                    warm[0:CR + 1, 0:455], vt1_sb[:, 0, :], vt1_sb[:, 8:15, :],
                    start=True, stop=True,
                )

            # one descriptor for all weights, then xf streamed in
            # consumption order so projections start after ~0.4MB, not 4MB.
            nc.sync.dma_start(wpack_sb[:], wpack_d[:])
            for ch in range(2):  # first 512 cols: everything pair 0 needs
                nc.sync.dma_start(xf_sb[:, ch, 0:512], xf_d[ch, :, 0:512])
            for j in range(7):  # rest of xf in half-MB pieces, both chunks
                for ch in range(2):
                    nc.sync.dma_start(
                        xf_sb[:, ch, 512 + j * 512 : 1024 + j * 512],
                        xf_d[ch, :, 512 + j * 512 : 1024 + j * 512],
                    )

            # query tiles (half-width final tiles measured slower: the extra
            # per-ACTIVATE overhead outweighs the shorter exposed tail)
            TILES = [(0, 512), (512, 512), (1024, 512), (1536, 512)]
            q_proj(*TILES[0])

            # ---- attention: tails software-pipelined one iteration behind
            # so the PE FIFO never stalls on the DVE normalization chain;
            # k and vT projections stream just-in-time inside tile 0. ----
            oTs, xrss = {}, {}
            for t, (qoff, qw) in enumerate(TILES):
                # prefetch residual tiles for this tile's tail; on the Sync
                # queue so they issue AFTER the xf pieces (HBM bandwidth
                # during setup is the critical resource)
                xrs = []
                for ch in range(2):
                    xr = outp.tile([128, qw], f32, tag="xr", name="xr")
                    nc.sync.dma_start(xr[:], xqf_d[ch, :, qoff : qoff + qw])
                    xrs.append(xr)
                xrss[t] = xrs
                oT = opool.tile([CR + 1, qw], f32, tag="ot", name="ot")
                oTs[t] = oT
                pairs(qoff, qw, oT, interleave_proj=(t == 0))
                if t + 1 < len(TILES):
                    q_proj(*TILES[t + 1])
                if t >= 1:
                    tail(*TILES[t - 1], oTs.pop(t - 1), xrss.pop(t - 1))
            accum_flush()
            last_i = len(TILES) - 1
            tail(*TILES[last_i], oTs.pop(last_i), xrss.pop(last_i), last=True)

        for _rep in range(reps):
            body()

    with tile.TileContext(nc) as tc:
        kern(tc)
    nc.compile()
    return nc


def _get_program(reps=1):
    key = ("nc", reps)
    if key not in _CACHE:
        _CACHE[key] = _build_program(reps)
    return _CACHE[key]


def _make_in_maps(x, Wq, Wk, Wv, Wp):
    bf16 = ml_dtypes.bfloat16
    xfull = np.ascontiguousarray(x.reshape(B, C, N))
    wpack = np.zeros((128, 768), dtype=bf16)
    wq2 = np.concatenate([Wq.T, Wq.T], axis=1)  # [256, 128]
    for ch in range(2):
        wpack[:, ch * 128 : (ch + 1) * 128] = wq2[ch * 128 : (ch + 1) * 128]
        wpack[:, 256 + ch * CR : 256 + (ch + 1) * CR] = Wk.T[ch * 128 : (ch + 1) * 128]
        wpack[:, 384 + ch * CR : 384 + (ch + 1) * CR] = Wv.T[ch * 128 : (ch + 1) * 128]
    wpack[0:CR, 512:768] = Wp.T
    in_maps = []
    for c in range(NCORES):
        b, h = divmod(c, 2)
        xb = xfull[b]
        xqs = np.ascontiguousarray(xb[:, h * NQ : (h + 1) * NQ])
        # put the core's query half first; key order is irrelevant to attention
        xperm = np.concatenate([xqs, xb[:, (1 - h) * NQ : (2 - h) * NQ]], axis=1)
        in_maps.append(
            {
                "xf": xperm.reshape(2, 128, N).astype(bf16),
                "xqf": xqs.reshape(2, 128, NQ).astype(np.float32),
                "wpack": wpack,
            }
        )
    return in_maps


def _run(x, Wq, Wk, Wv, Wp):
    from concourse import bass_utils

    nc = _get_program()
    in_maps = _make_in_maps(x, Wq, Wk, Wv, Wp)
    res = bass_utils.run_bass_kernel_spmd(nc, in_maps, core_ids=list(range(NCORES)))
    out = np.empty((B, C, N), dtype=np.float32)
    for c in range(NCORES):
        b, h = divmod(c, 2)
        out[b, :, h * NQ : (h + 1) * NQ] = res.results[c]["out"].reshape(C, NQ)
    return out.reshape(B, C, HH, WW)


def kernel(x, Wq, Wk, Wv, Wp):
    return _run(
        np.asarray(x, dtype=np.float32),
        np.asarray(Wq, dtype=np.float32),
        np.asarray(Wk, dtype=np.float32),
        np.asarray(Wv, dtype=np.float32),
        np.asarray(Wp, dtype=np.float32),
    )


# ---------------------------------------------------------------------------
# benchmarking helpers (not used by the grading path)
# ---------------------------------------------------------------------------


def _get_exec(reps):
    """Build a cached jitted shard_map executable for the given reps-variant
    (mirrors bass2jax.run_bass_via_pjrt, but reusable across calls)."""
    key = ("exec", reps)
    if key in _CACHE:
        return _CACHE[key]

    import jax
    from jax.experimental.shard_map import shard_map
    from jax.sharding import Mesh, PartitionSpec
    import concourse.mybir as mybir
    from concourse.bass2jax import (
        _bass_exec_p,
        install_neuronx_cc_hook,
        partition_id_tensor,
    )

    install_neuronx_cc_hook()
    nc = _get_program(reps)
    partition_name = nc.partition_id_tensor.name if nc.partition_id_tensor else None

    in_names, out_names, out_avals, zero_outs = [], [], [], []
    for alloc in nc.m.functions[0].allocations:
        if not isinstance(alloc, mybir.MemoryLocationSet):
            continue
        name = alloc.memorylocations[0].name
        if alloc.kind == "ExternalInput":
            if name != partition_name:
                in_names.append(name)
        elif alloc.kind == "ExternalOutput":
            out_names.append(name)
            shape = tuple(alloc.tensor_shape)
            dtype = mybir.dt.np(alloc.dtype)
            out_avals.append(jax.core.ShapedArray(shape, dtype))
            zero_outs.append(np.zeros(shape, dtype))
    n_params = len(in_names)
    n_outs = len(out_avals)
    all_in_names = in_names + out_names
    if partition_name is not None:
        all_in_names.append(partition_name)
    donate = tuple(range(n_params, n_params + n_outs))

    def _b(*args):
        operands = list(args)
        if partition_name is not None:
            operands.append(partition_id_tensor())
        outs = _bass_exec_p.bind(
            *operands,
            out_avals=tuple(out_avals),
            in_names=tuple(all_in_names),
            out_names=tuple(out_names),
            lowering_input_output_aliases=(),
            sim_require_finite=True,
            sim_require_nnan=True,
            nc=nc,
        )
        return tuple(outs)

    devices = jax.devices()[:NCORES]
    mesh = Mesh(np.asarray(devices), ("core",))
    in_specs = (PartitionSpec("core"),) * (n_params + n_outs)
    out_specs = (PartitionSpec("core"),) * n_outs
    fn = jax.jit(
        shard_map(_b, mesh=mesh, in_specs=in_specs, out_specs=out_specs, check_rep=False),
        donate_argnums=donate,
        keep_unused=True,
    )
    _CACHE[key] = (fn, in_names, out_names, out_avals, zero_outs, mesh)
    return _CACHE[key]


def bench(x, Wq, Wk, Wv, Wp, reps, iters=8):
    """Return (best_wall_seconds, outputs_list) for the reps-variant program."""
    import time

    import jax

    fn, in_names, out_names, out_avals, zero_outs, mesh = _get_exec(reps)
    in_maps = _make_in_maps(x, Wq, Wk, Wv, Wp)
    concat_in = [
        np.concatenate([in_maps[c][n] for c in range(NCORES)], axis=0)
        for n in in_names
    ]
    concat_in = [jax.device_put(a) for a in concat_in]

    def zeros():
        return [np.zeros((NCORES * z.shape[0], *z.shape[1:]), z.dtype) for z in zero_outs]

    # warm up (compiles NEFF on first call)
    out = fn(*concat_in, *zeros())
    jax.block_until_ready(out)

    best = float("inf")
    for _ in range(iters):
        zs = [jax.device_put(z) for z in zeros()]
        jax.block_until_ready(zs)
        t0 = time.perf_counter()
        out = fn(*concat_in, *zs)
        jax.block_until_ready(out)
        t1 = time.perf_counter()
        best = min(best, t1 - t0)
    outs = [np.asarray(o) for o in out]
    return best, outs

